# revision 21
# baseline (speedup 1.0000x reference)
"""GeneralSampleEdgeConv Trainium2 kernel, 8-core SPMD — Plan B.

out = segment_sum(mask * (node_feature[src] ++ edge_feature) @ W_msg, dst)

The call is dominated by host->device staging over the axon tunnel, so the
kernel minimizes shipped bytes:
  - edge features: fp8(e3m4), host-bucketed by dst tile (one-hot segment
    sum on device needs no per-edge index upload beyond 1 dst_rel lane);
  - node features: uploaded SHARDED (N/8 rows per core, fp8), AllGather'd
    on device, expanded to a 256B-stride HBM table, and x_j slabs gathered
    on device with gpsimd.dma_gather using baked int16 indices (edges are
    split per dst tile into src<32768 / src>=32768 groups because gather
    indices are int16; <=1024 idxs per call — larger crashes SWDGE);
  - output: int8 with device-computed per-row scales, dequantized on host.
Per dst tile the device one-hot-accumulates psum[128,96] over chunks,
transposes, projects with W_msg halves, quantizes, and stores.
"""
import math
import numpy as np

import concourse.tile as tile
from concourse import bass, bacc, mybir

F16 = mybir.dt.float16
F32 = mybir.dt.float32
F8 = mybir.dt.float8e3  # e3m4
I16 = mybir.dt.int16
I8 = mybir.dt.int8
U8 = mybir.dt.uint8

# Problem geometry (full size; sim mode overrides these module globals)
N, E, D = 50000, 800000, 96
PT = 128
NCORES = 8
SPLIT = 32768           # int16-index split for the gather table
SEG = 64                # ef chunks per DMA slab
SEGX = 64               # xj chunks per gather slab


def _geom():
    nt = math.ceil(N / PT)
    slots = math.ceil(nt / NCORES)
    return nt, slots, slots * NCORES, N // NCORES


def _build(cc_ef, cl, ch, for_sim=False):
    """cc_ef[s]: ef chunks for slot s; cl[s]/ch[s]: low/high xj chunks."""
    nt, SLOTS, NTP, NSH = _geom()
    CTE = int(sum(cc_ef))
    L, H = int(sum(cl)), int(sum(ch))
    CTX = L + H
    EO = np.concatenate([[0], np.cumsum(cc_ef)]).astype(int)
    LO = np.concatenate([[0], np.cumsum(cl)]).astype(int)
    HO = np.concatenate([[0], np.cumsum(ch)]).astype(int)

    # consts (f16): ident 128 | wt 96 | wb 96.  drel (u8): iota 128 | dre | drx
    WT0, WB0 = 128, 224
    CW = 320
    DRE0 = 128
    DRX0 = DRE0 + CTE
    DW = DRX0 + CTX
    GI = CTX * 8

    if for_sim:
        nc = bacc.Bacc("TRN2", num_devices=NCORES,
                       target_bir_lowering=False, debug=True)
    else:
        nc = bacc.Bacc("TRN2", num_devices=NCORES)
    ef = nc.dram_tensor("ef", [128, CTE * 96], F8, kind="ExternalInput")
    gidx = nc.dram_tensor("gidx", [16, GI], I16, kind="ExternalInput")
    nfsh = nc.dram_tensor("nfsh", [NSH, 96], F8, kind="ExternalInput")
    consts = nc.dram_tensor("consts", [128, CW], F16, kind="ExternalInput")
    drel = nc.dram_tensor("drel", [128, DW], U8, kind="ExternalInput")
    out = nc.dram_tensor("out", [SLOTS * PT, D], I8, kind="ExternalOutput")
    oscale = nc.dram_tensor("oscale", [128, SLOTS], F32, kind="ExternalOutput")

    with tile.TileContext(nc) as tc:
        with (
            tc.tile_pool(name="const", bufs=1) as constp,
            tc.tile_pool(name="dram", bufs=1, space="DRAM") as dram,
            tc.tile_pool(name="efslab", bufs=3) as efslabp,
            tc.tile_pool(name="xslab", bufs=3) as xslabp,
            tc.tile_pool(name="sb", bufs=3) as sb,
            tc.tile_pool(name="eplg", bufs=2) as ep,
            tc.tile_pool(name="psa", bufs=2, space="PSUM") as psa,
            tc.tile_pool(name="psb", bufs=2, space="PSUM") as psb,
            tc.tile_pool(name="pst", bufs=1, space="PSUM") as pst,
            tc.tile_pool(name="pso", bufs=2, space="PSUM") as pso,
        ):
            ccst = constp.tile([128, CW], F16)
            nc.sync.dma_start(out=ccst[:], in_=consts[:, :])
            ident = ccst[:, 0:128]
            wt_sb = ccst[0:96, WT0:WT0 + 96]
            wb_sb = ccst[0:96, WB0:WB0 + 96]
            drt = constp.tile([128, DW], U8)
            nc.sync.dma_start(out=drt[:], in_=drel[:, :])
            iota_u8 = drt[:, 0:128]
            scl = constp.tile([128, SLOTS], F32)

            gt = constp.tile([128, GI], I16)
            for k in range(8):
                nc.sync.dma_start(out=gt[k * 16:(k + 1) * 16, :], in_=gidx[:, :])

            # node table: compact fp8 shard -> AllGather -> 256B-stride table
            # (table cols 96:256 stay uninitialized; gathered but never read)
            cc_in = dram.tile([NSH, 96], F8, tag="ccin")
            cc_out = dram.tile([NCORES * NSH, 96], F8, tag="ccout",
                               addr_space="Shared")
            table = dram.tile([NCORES * NSH, 256], F8, tag="table")
            nc.sync.dma_start(out=cc_in[:, :], in_=nfsh[:, :])
            nc.gpsimd.collective_compute(
                "AllGather", mybir.AluOpType.bypass,
                replica_groups=[list(range(NCORES))],
                ins=[cc_in[:, :]], outs=[cc_out[:, :]],
            )
            # strided pad-expand: split across DMA queues for parallelism
            NR = NCORES * NSH
            for k in range(4):
                r0, r1 = k * NR // 4, (k + 1) * NR // 4
                nc.sync.dma_start(out=table[r0:r1, 0:96], in_=cc_out[r0:r1, :])

            efslabs, xslabs = {}, {}

            def ef_of(c):
                k = c // SEG
                if k not in efslabs:
                    nch = min(SEG, CTE - k * SEG)
                    t = efslabp.tile([128, SEG * 96], F8, tag="efslab")
                    nc.sync.dma_start(
                        out=t[:, : nch * 96],
                        in_=ef[:, k * SEG * 96:(k * SEG + nch) * 96],
                    )
                    efslabs[k] = t
                return efslabs[k], c - k * SEG

            GB = 8  # chunks per dma_gather call (1024 idxs; >=2048 crashes SWDGE)

            def xj_of(cx):
                k = cx // SEGX
                if k not in xslabs:
                    j0, j1 = k * SEGX, min(CTX, (k + 1) * SEGX)
                    t = xslabp.tile([128, SEGX, 256], F8, tag="xslab")
                    for a0, b0, base_lo in ((max(j0, 0), min(j1, L), True),
                                            (max(j0, L), min(j1, CTX), False)):
                        src_ap = table[0:SPLIT, :] if base_lo else table[SPLIT:N, :]
                        for a in range(a0, b0, GB):
                            b = min(b0, a + GB)
                            nc.gpsimd.dma_gather(
                                out_ap=t[:, a - j0:b - j0, :],
                                in_ap=src_ap,
                                idxs_ap=gt[:, a * 8:b * 8],
                                num_idxs=(b - a) * 128,
                                num_idxs_reg=(b - a) * 128,
                                elem_size=256,
                            )
                    xslabs[k] = t
                return xslabs[k], cx - k * SEGX

            for s in range(SLOTS):
                pa = psa.tile([128, 96], F32, tag="pa")
                pb = psb.tile([128, 96], F32, tag="pb")

                # x_j side: low chunks then high chunks of this slot
                xcs = ([LO[s] + j for j in range(int(cl[s]))]
                       + [L + HO[s] + j for j in range(int(ch[s]))])
                for i, cx in enumerate(xcs):
                    slab, lc = xj_of(cx)
                    P = sb.tile([128, 128], F8, tag="onehot_x")
                    nc.vector.tensor_tensor(
                        out=P[:],
                        in0=drt[:, DRX0 + cx:DRX0 + cx + 1].to_broadcast([128, 128]),
                        in1=iota_u8,
                        op=mybir.AluOpType.is_equal,
                    )
                    nc.tensor.matmul(
                        out=pa[:], lhsT=P[:], rhs=slab[:, lc, 0:96],
                        start=(i == 0), stop=(i == len(xcs) - 1),
                    )

                # edge-feature side
                nce = int(cc_ef[s])
                for j in range(nce):
                    c = EO[s] + j
                    slab, lc = ef_of(c)
                    P = sb.tile([128, 128], F8, tag="onehot_e")
                    nc.vector.tensor_tensor(
                        out=P[:],
                        in0=drt[:, DRE0 + c:DRE0 + c + 1].to_broadcast([128, 128]),
                        in1=iota_u8,
                        op=mybir.AluOpType.is_equal,
                    )
                    nc.tensor.matmul(
                        out=pb[:], lhsT=P[:], rhs=slab[:, lc * 96:lc * 96 + 96],
                        start=(j == 0), stop=(j == nce - 1),
                    )

                a16 = ep.tile([128, 96], F16, tag="a16")
                nc.vector.tensor_copy(out=a16[:], in_=pa[:])
                b16 = ep.tile([128, 96], F16, tag="b16")
                nc.vector.tensor_copy(out=b16[:], in_=pb[:])
                tpa = pst.tile([96, 128], F16, tag="tpa")
                nc.tensor.transpose(out=tpa[:], in_=a16[:], identity=ident)
                tpb = pst.tile([96, 128], F16, tag="tpb")
                nc.tensor.transpose(out=tpb[:], in_=b16[:], identity=ident)
                aT = ep.tile([96, 128], F16, tag="aT")
                nc.vector.tensor_copy(out=aT[:], in_=tpa[:])
                bT = ep.tile([96, 128], F16, tag="bT")
                nc.vector.tensor_copy(out=bT[:], in_=tpb[:])
                ops = pso.tile([128, 96], F32, tag="ops")
                nc.tensor.matmul(out=ops[:], lhsT=aT[:], rhs=wt_sb, start=True, stop=False)
                nc.tensor.matmul(out=ops[:], lhsT=bT[:], rhs=wb_sb, start=False, stop=True)

                # int8 quantization with per-row scale: q = v * 127/max|row|
                rmax = ep.tile([128, 1], F32, tag="rmax")
                nc.vector.tensor_reduce(
                    out=rmax[:], in_=ops[:], axis=mybir.AxisListType.X,
                    op=mybir.AluOpType.max, apply_absolute_value=True)
                nc.vector.tensor_scalar_max(
                    out=scl[:, s:s + 1], in0=rmax[:], scalar1=1e-30)
                rinv = ep.tile([128, 1], F32, tag="rinv")
                nc.vector.reciprocal(out=rinv[:], in_=scl[:, s:s + 1])
                rs = ep.tile([128, 1], F32, tag="rs")
                nc.vector.tensor_scalar_mul(out=rs[:], in0=rinv[:], scalar1=127.0)
                oi8 = ep.tile([128, 96], I8, tag="oi8")
                nc.vector.tensor_tensor(
                    out=oi8[:], in0=ops[:], in1=rs[:].to_broadcast([128, 96]),
                    op=mybir.AluOpType.mult)
                nc.sync.dma_start(out=out[s * PT:(s + 1) * PT, :], in_=oi8[:])
            nc.sync.dma_start(out=oscale[:, :], in_=scl[:])
    nc.compile()
    return nc


def _prep(node_feature, edge_feature, edge_index, edge_mask):
    """Returns (cc_ef, cl, ch, in_maps_payload, tiles_of_core)."""
    import ml_dtypes

    f8 = ml_dtypes.float8_e3m4
    nt, SLOTS, NTP, NSH = _geom()
    src = np.asarray(edge_index[0], dtype=np.int64)
    dst = np.asarray(edge_index[1], dtype=np.int64)
    keep = np.asarray(edge_mask, dtype=bool)
    src, dst = src[keep], dst[keep]
    ef = np.asarray(edge_feature, dtype=np.float32)[keep].astype(f8)
    nf16 = np.asarray(node_feature, dtype=np.float32).astype(np.float16)

    tid = dst >> 7
    # order edges by (tile, src-range) so each tile's low group precedes high
    hi = (src >= SPLIT).astype(np.int64)
    order = np.argsort(tid * 2 + hi, kind="stable")
    src, dst, ef, tid, hi = src[order], dst[order], ef[order], tid[order], hi[order]
    cnt = np.zeros(NTP, np.int64)
    np.add.at(cnt, tid, 1)
    cntl = np.zeros(NTP, np.int64)
    np.add.at(cntl, tid, 1 - hi)
    cnth = cnt - cntl
    starts = np.concatenate([[0], np.cumsum(cnt)])

    # snake-deal tiles (desc count) to cores
    rank = np.argsort(-cnt[:NTP], kind="stable")
    tiles_of_core = [[] for _ in range(NCORES)]
    for r, t in enumerate(rank):
        blk, pos = divmod(r, NCORES)
        c = pos if blk % 2 == 0 else NCORES - 1 - pos
        tiles_of_core[c].append(int(t))

    cc_ef = np.ones(SLOTS, np.int64)
    cl = np.zeros(SLOTS, np.int64)
    ch = np.zeros(SLOTS, np.int64)
    for s in range(SLOTS):
        ts = [tiles_of_core[c][s] for c in range(NCORES)]
        cc_ef[s] = max(1, max(math.ceil(cnt[t] / PT) for t in ts))
        cl[s] = max(math.ceil(cntl[t] / PT) for t in ts)
        ch[s] = max(math.ceil(cnth[t] / PT) for t in ts)
        if cl[s] + ch[s] == 0:
            cl[s] = 1
    CTE = int(cc_ef.sum())
    L, H = int(cl.sum()), int(ch.sum())
    CTX = L + H
    EO = np.concatenate([[0], np.cumsum(cc_ef)]).astype(int)
    LO = np.concatenate([[0], np.cumsum(cl)]).astype(int)
    HO = np.concatenate([[0], np.cumsum(ch)]).astype(int)

    payload = []
    for c in range(NCORES):
        efa = np.zeros((CTE * PT, 96), f8)
        dre = np.full(CTE * PT, 255, np.uint8)   # 255 = pad (matches no iota)
        drx = np.full(CTX * PT, 255, np.uint8)
        gix = np.zeros(CTX * PT, np.int16)
        for s in range(SLOTS):
            t = tiles_of_core[c][s]
            e0 = starts[t]
            nl, nh = int(cntl[t]), int(cnth[t])
            n = nl + nh
            # ef layout: tile-contiguous chunks
            o = EO[s] * PT
            efa[o:o + n] = ef[e0:e0 + n]
            dre[o:o + n] = (dst[e0:e0 + n] - t * PT).astype(np.uint8)
            # xj layout: low region chunks
            o = LO[s] * PT
            gix[o:o + nl] = src[e0:e0 + nl].astype(np.int16)
            drx[o:o + nl] = (dst[e0:e0 + nl] - t * PT).astype(np.uint8)
            # xj layout: high region chunks
            o = (L + HO[s]) * PT
            gix[o:o + nh] = (src[e0 + nl:e0 + n] - SPLIT).astype(np.int16)
            drx[o:o + nh] = (dst[e0 + nl:e0 + n] - t * PT).astype(np.uint8)
        efa = np.ascontiguousarray(
            efa.reshape(CTE, PT, 96).transpose(1, 0, 2).reshape(PT, CTE * 96))
        gixw = np.ascontiguousarray(gix.reshape(CTX * 8, 16).T)  # [16, CTX*8]
        payload.append({
            "ef": efa,
            "gidx": gixw,
            "nfsh": np.ascontiguousarray(nf16[c * NSH:(c + 1) * NSH].astype(f8)),
            "dre": np.ascontiguousarray(dre.reshape(CTE, PT).T),
            "drx": np.ascontiguousarray(drx.reshape(CTX, PT).T),
        })
    return cc_ef, cl, ch, payload, tiles_of_core


def _in_maps(payload, W_msg):
    w16 = np.asarray(W_msg, dtype=np.float32).astype(np.float16)
    iota8 = np.tile(np.arange(128, dtype=np.uint8), (128, 1))
    ident = np.eye(128, dtype=np.float16)
    wt = np.zeros((128, 96), np.float16); wt[:96] = w16[:96]
    wb = np.zeros((128, 96), np.float16); wb[:96] = w16[96:]
    consts = np.concatenate([ident, wt, wb], axis=1)
    in_maps = []
    for p in payload:
        dr = np.concatenate([iota8, p["dre"], p["drx"]], axis=1)
        in_maps.append({"ef": p["ef"], "gidx": p["gidx"],
                        "nfsh": p["nfsh"], "consts": consts, "drel": dr})
    return in_maps


def kernel(node_feature, edge_feature, edge_index, edge_mask, W_msg):
    from concourse.bass_utils import run_bass_kernel_spmd

    nt, SLOTS, NTP, NSH = _geom()
    cc_ef, cl, ch, payload, tiles_of_core = _prep(
        node_feature, edge_feature, edge_index, edge_mask)
    nc = _build(cc_ef, cl, ch)
    in_maps = _in_maps(payload, W_msg)
    res = run_bass_kernel_spmd(nc, in_maps, list(range(NCORES)))

    out_full = np.zeros((NTP * PT, D), np.float32)
    for c in range(NCORES):
        oc = np.asarray(res.results[c]["out"]).astype(np.float32)
        osc = np.asarray(res.results[c]["oscale"], dtype=np.float32)
        for s in range(SLOTS):
            t = tiles_of_core[c][s]
            out_full[t * PT:(t + 1) * PT] = (
                oc[s * PT:(s + 1) * PT] * (osc[:, s:s + 1] / 127.0))
    return out_full[:N]
